# revision 10
# baseline (speedup 1.0000x reference)
"""CoefficientMaxPool Trainium2 kernel (8-core data-parallel).

Problem: x [32, 512, 16, 128] f32.  Irreps group into degree blocks
l=0:[0,1), l=1:[1,4), l=2:[4,9), l=3:[9,16).  Per (batch, l, channel):
find the neighbor n* maximizing the degree-block squared norm, output
that neighbor's block components -> out [32, 16, 128].

Per core (4 batches), per batch:
  - DMA x[b] as [p=128, a=4, i=16, c=128] (n = a*128 + p)
  - ACT: x2 = x*x
  - GPSIMD: norms l1 (adds); DVE: norms l2/l3 (grouped reduces)
  - DVE: amax = max over a (tensor_tensor max tree)
  - PE: transpose amax -> [c, l, p] PSUM; DVE reduce-max over p -> maxv[c,l];
        PE transpose -> [1, l, c]; K=1 ones matmul broadcasts to maxB[p,l,c]
  - DVE: mask = (norms == maxB)  (exact equality, unique winner - verified)
  - DVE (l0..l2) + GPSIMD (l3): x *= mask[l(i)]  in place
  - PE: ones[128,1]^T @ x accumulated over a -> out[1, i*c] in PSUM
  - ACT copy PSUM->SBUF, DMA out.
"""

import os
import sys

import numpy as np

for _p in ("/opt/trn_rl_repo", "/opt/pypackages"):
    if _p not in sys.path:
        sys.path.append(_p)

from contextlib import ExitStack

import concourse.bacc as bacc
import concourse.bass as bass
import concourse.tile as tile
from concourse import mybir

N_CORES = 8
B_FULL, N, IRR, C = 32, 512, 16, 128
B = B_FULL // N_CORES  # 4 batches per core
P = 128                # partitions (n within chunk)
A = N // P             # 4 neighbor chunks
BLOCKS = [(0, 1), (1, 4), (4, 9), (9, 16)]  # irrep ranges per degree l
F32 = mybir.dt.float32
ADD = mybir.AluOpType.add
MAX = mybir.AluOpType.max
MULT = mybir.AluOpType.mult
EQ = mybir.AluOpType.is_equal

_cache = {}


def _build_bass():
    nc = bacc.Bacc("TRN2", target_bir_lowering=False, debug=False,
                   num_devices=N_CORES)
    x_in = nc.dram_tensor("x", [B, N, IRR, C], F32, kind="ExternalInput")
    out_t = nc.dram_tensor("out", [B, IRR, C], F32, kind="ExternalOutput")
    ident_d = nc.inline_tensor(np.eye(P, dtype=np.float32), name="ident")

    with tile.TileContext(nc) as tc, ExitStack() as ctx:
        # DRAM view: n = a*P + p  ->  [b, p, a, i, c]
        x_v = x_in.ap().rearrange("b (a p) i c -> b p a i c", p=P)
        out_v = out_t.ap().rearrange("b i c -> (b i c)").unsqueeze(0)

        xp = ctx.enter_context(tc.tile_pool(name="xp", bufs=2))
        x2p = ctx.enter_context(tc.tile_pool(name="x2p", bufs=2))
        med = ctx.enter_context(tc.tile_pool(name="med", bufs=2))
        outp = ctx.enter_context(tc.tile_pool(name="outp", bufs=2))
        singles = ctx.enter_context(tc.tile_pool(name="singles", bufs=1))
        pmax = ctx.enter_context(tc.tile_pool(name="pmax", bufs=2, space="PSUM"))
        pout = ctx.enter_context(tc.tile_pool(name="pout", bufs=1, space="PSUM"))
        dscr = ctx.enter_context(tc.tile_pool(name="dscr", bufs=2, space="DRAM"))

        ones = singles.tile([P, 1], F32)
        nc.vector.memset(ones, 1.0)
        onesrow = singles.tile([1, P], F32)
        nc.vector.memset(onesrow, 1.0)
        ident = singles.tile([P, P], F32)
        nc.sync.dma_start(out=ident, in_=ident_d.ap())

        for b in range(B):
            X = xp.tile([P, A, IRR, C], F32, tag="X")
            nc.sync.dma_start(out=X, in_=x_v[b])

            X2 = x2p.tile([P, A, IRR, C], F32, tag="X2")
            nc.scalar.activation(X2, X, mybir.ActivationFunctionType.Square)

            # grouped sums over i -> norms [P, A, 3, C] for l=1..3
            # (l=0 norm is X2[:, :, 0, :] itself)
            norms = med.tile([P, A, 3, C], F32, tag="norms")
            # grouped reduces (i innermost via AP)
            for j, (s, e) in ((0, BLOCKS[1]), (1, BLOCKS[2]), (2, BLOCKS[3])):
                nc.vector.tensor_reduce(
                    out=norms[:, :, j, :],
                    in_=X2[:, :, s:e, :].rearrange("p a i c -> p a c i"),
                    axis=mybir.AxisListType.X,
                    op=ADD,
                )

            # max over a -> amax [P, 4, C] via tensor_tensor max trees
            amax = med.tile([P, 4, C], F32, tag="amax")
            t0 = med.tile([P, C], F32, tag="t0")
            nc.vector.tensor_tensor(t0, X2[:, 0, 0, :], X2[:, 1, 0, :], MAX)
            nc.vector.tensor_tensor(
                amax[:, 0, :], X2[:, 2, 0, :], X2[:, 3, 0, :], MAX)
            nc.vector.tensor_tensor(amax[:, 0, :], amax[:, 0, :], t0, MAX)
            t1 = med.tile([P, 3, C], F32, tag="t1")
            nc.vector.tensor_tensor(t1, norms[:, 0], norms[:, 1], MAX)
            nc.vector.tensor_tensor(amax[:, 1:4, :], norms[:, 2], norms[:, 3], MAX)
            nc.vector.tensor_tensor(amax[:, 1:4, :], amax[:, 1:4, :], t1, MAX)

            # cross-partition max: transpose [p, c] -> [c, p] per l, reduce
            nt = pmax.tile([P, 4, P], F32, tag="nt")  # [c, l, p]
            for l in range(4):
                nc.tensor.transpose(nt[:, l, :], amax[:, l, :], ident)
            maxv = med.tile([P, 4], F32, tag="maxv")  # [c, l]
            nc.vector.tensor_reduce(
                out=maxv, in_=nt, axis=mybir.AxisListType.X, op=MAX)
            # [c, l] -> [1, l, c]; broadcast to all partitions via K=1 matmul
            mvt = pmax.tile([1, 4, P], F32, tag="mvt")
            for l in range(4):
                nc.tensor.transpose(mvt[:, l, :], maxv[:, l:l + 1], ident)
            mvts = med.tile([1, 4, P], F32, tag="mvts")
            nc.scalar.copy(out=mvts, in_=mvt)
            # broadcast [1, l, c] -> [p, l, c]: bounce via DRAM, then
            # replicate on the DRAM->SBUF read (partition stride 0)
            mvd = dscr.tile([1, 4, P], F32, tag="mvd")
            nc.sync.dma_start(out=mvd, in_=mvts)
            maxB = med.tile([P, 4, C], F32, tag="maxB")
            mvd_rep = bass.AP(
                tensor=mvd.tensor,
                offset=mvd.offset,
                ap=[[0, P]] + list(mvd.ap[1:]),
            )
            nc.gpsimd.dma_start(out=maxB, in_=mvd_rep)

            # winner mask (exact equality; unique winner)
            mask = med.tile([P, A, 4, C], F32, tag="mask")
            nc.vector.tensor_tensor(
                mask[:, :, 0, :],
                X2[:, :, 0, :],
                maxB[:, 0:1, :].broadcast_to([P, A, C]),
                EQ,
            )
            nc.vector.tensor_tensor(
                mask[:, :, 1:4, :],
                norms,
                maxB[:, 1:4, :].unsqueeze(1).broadcast_to([P, A, 3, C]),
                EQ,
            )

            # select winner values in place: X *= mask[l(i)]
            for l, (s, e) in enumerate(BLOCKS):
                nc.vector.tensor_tensor(
                    X[:, :, s:e, :],
                    X[:, :, s:e, :],
                    mask[:, :, l, :].unsqueeze(2).broadcast_to([P, A, e - s, C]),
                    MULT,
                )

            # sum over n (partitions via PE, chunks via PSUM accumulation)
            Xf = X.rearrange("p a i c -> p a (i c)")
            ob = outp.tile([1, IRR * C], F32, tag="ob")
            for h in range(2):
                ps = pout.tile([1, 2, 512], F32, tag="ps")
                for kk in range(2):
                    k = h * 2 + kk
                    for a in range(A):
                        nc.tensor.matmul(
                            ps[:, kk, :],
                            ones,
                            Xf[:, a, k * 512:(k + 1) * 512],
                            start=(a == 0),
                            stop=(a == A - 1),
                        )
                nc.scalar.copy(out=ob[:, h * 1024:(h + 1) * 1024],
                               in_=ps.rearrange("m k f -> m (k f)"))
            nc.sync.dma_start(out=out_v[:, b * IRR * C:(b + 1) * IRR * C], in_=ob)

    nc.compile()
    return nc


def kernel(x: np.ndarray, i2l: np.ndarray | None = None) -> np.ndarray:
    x = np.ascontiguousarray(np.asarray(x), dtype=np.float32)
    assert x.shape == (B_FULL, N, IRR, C), x.shape

    if "nc" not in _cache:
        _cache["nc"] = _build_bass()
    nc = _cache["nc"]

    from concourse.bass_utils import run_bass_kernel_spmd

    in_maps = [{"x": x[i * B:(i + 1) * B]} for i in range(N_CORES)]
    res = run_bass_kernel_spmd(nc, in_maps, list(range(N_CORES)))
    out = np.concatenate([res.results[i]["out"] for i in range(N_CORES)], axis=0)
    return out


if __name__ == "__main__":
    xs = np.random.randn(B_FULL, N, IRR, C).astype(np.float32)
    o = kernel(xs)
    print("out", o.shape, o.dtype)


# revision 12
# speedup vs baseline: 1.3535x; 1.3535x over previous
"""CoefficientMaxPool Trainium2 kernel (8-core data-parallel).

Problem: x [32, 512, 16, 128] f32.  Irreps group into degree blocks
l=0:[0,1), l=1:[1,4), l=2:[4,9), l=3:[9,16).  Per (batch, l, channel):
find the neighbor n* maximizing the degree-block squared norm, output
that neighbor's block components -> out [32, 16, 128].

Per core (4 batches), per batch (transposed-norms architecture):
  - DMA x[b] as [p=128, a=4, i=16, c=128] (n = a*128 + p)
  - ACT: x2 = x*x
  - Degree norms land TRANSPOSED in PSUM as NT_l [c, n]:
      l0: PE transposes of x2[:, a, 0, :]
      l2: PE transpose-accumulation over i in [4, 9)
      l1/l3: DVE grouped reduce (n-side) then PE transpose
  - DVE: maxv_l[c] = reduce-max over n (free dim);  maskT_l[c, n] =
    (NT_l == maxv_l) via tensor_scalar is_equal (per-partition scalar)
  - PE: transpose maskT_l back -> MP_l [p(chunk), a, c] in PSUM
  - DVE: x *= MP[l(i)]  in place (winner-select; exact, unique winner)
  - PE: ones[128,1]^T @ x accumulated over a -> out[1, i*c] in PSUM
  - ACT copy PSUM->SBUF, DMA out.
"""

import os
import sys

import numpy as np

for _p in ("/opt/trn_rl_repo", "/opt/pypackages"):
    if _p not in sys.path:
        sys.path.append(_p)

from contextlib import ExitStack

import concourse.bacc as bacc
import concourse.bass as bass
import concourse.tile as tile
from concourse import mybir

N_CORES = 8
B_FULL, N, IRR, C = 32, 512, 16, 128
B = B_FULL // N_CORES  # 4 batches per core
P = 128                # partitions (n within chunk)
A = N // P             # 4 neighbor chunks
BLOCKS = [(0, 1), (1, 4), (4, 9), (9, 16)]  # irrep ranges per degree l
F32 = mybir.dt.float32
ADD = mybir.AluOpType.add
MAX = mybir.AluOpType.max
MULT = mybir.AluOpType.mult
EQ = mybir.AluOpType.is_equal

_cache = {}


def _build_bass():
    nc = bacc.Bacc("TRN2", target_bir_lowering=False, debug=False,
                   num_devices=N_CORES)
    x_in = nc.dram_tensor("x", [B, N, IRR, C], F32, kind="ExternalInput")
    out_t = nc.dram_tensor("out", [B, IRR, C], F32, kind="ExternalOutput")
    ident_d = nc.inline_tensor(np.eye(P, dtype=np.float32), name="ident")

    with tile.TileContext(nc) as tc, ExitStack() as ctx:
        # DRAM view: n = a*P + p  ->  [b, p, a, i, c]
        x_v = x_in.ap().rearrange("b (a p) i c -> b p a i c", p=P)
        out_v = out_t.ap().rearrange("b i c -> (b i c)").unsqueeze(0)

        xp = ctx.enter_context(tc.tile_pool(name="xp", bufs=2))
        x2p = ctx.enter_context(tc.tile_pool(name="x2p", bufs=2))
        med = ctx.enter_context(tc.tile_pool(name="med", bufs=2))
        outp = ctx.enter_context(tc.tile_pool(name="outp", bufs=2))
        singles = ctx.enter_context(tc.tile_pool(name="singles", bufs=1))
        # PSUM: one rotating pool of [128, 512] 1-bank tiles (NT_l / MP_l)
        psc = ctx.enter_context(tc.tile_pool(name="psc", bufs=4, space="PSUM"))
        pout = ctx.enter_context(tc.tile_pool(name="pout", bufs=2, space="PSUM"))

        ones = singles.tile([P, 1], F32)
        nc.vector.memset(ones, 1.0)
        ident = singles.tile([P, P], F32)
        nc.sync.dma_start(out=ident, in_=ident_d.ap())

        for b in range(B):
            X = xp.tile([P, A, IRR, C], F32, tag="X")
            nc.sync.dma_start(out=X, in_=x_v[b])

            X2 = x2p.tile([P, A, IRR, C], F32, tag="X2")
            nc.scalar.activation(X2, X, mybir.ActivationFunctionType.Square)

            # n-side grouped sums for l = 1..3 (DVE)
            norms = med.tile([P, A, 3, C], F32, tag="norms")
            for j, (s, e) in ((0, BLOCKS[1]), (1, BLOCKS[2]), (2, BLOCKS[3])):
                nc.vector.tensor_reduce(
                    out=norms[:, :, j, :],
                    in_=X2[:, :, s:e, :].rearrange("p a i c -> p a c i"),
                    axis=mybir.AxisListType.X, op=ADD)

            # transposed norms NT_l [c, n=a*128+p] in PSUM
            NT = []
            for l in range(4):
                nt = psc.tile([P, A, P], F32, tag="sc")
                NT.append(nt)
            for a in range(A):
                nc.tensor.transpose(NT[0][:, a, :], X2[:, a, 0, :], ident)
                for l in range(1, 4):
                    nc.tensor.transpose(NT[l][:, a, :], norms[:, a, l - 1, :],
                                        ident)

            # per-l: max over n, equality mask (transposed), transpose back
            MP = []
            for l in range(4):
                maxv = med.tile([P, 1], F32, tag=f"maxv{l}")
                nc.vector.tensor_reduce(
                    out=maxv, in_=NT[l], axis=mybir.AxisListType.XY, op=MAX)
                mt = med.tile([P, A, P], F32, tag=f"mt{l}")
                nc.vector.tensor_scalar(
                    out=mt, in0=NT[l], scalar1=maxv, scalar2=None, op0=EQ)
                mp = psc.tile([P, A, P], F32, tag="sc")
                for a in range(A):
                    nc.tensor.matmul(mp[:, a, :], mt[:, a, :], ident,
                                     is_transpose=True, start=True, stop=True)
                MP.append(mp)  # [p, a, c]

            # winner-select in place: X *= MP[l(i)]
            for l, (s, e) in enumerate(BLOCKS):
                nc.vector.tensor_tensor(
                    X[:, :, s:e, :],
                    X[:, :, s:e, :],
                    MP[l].unsqueeze(2).broadcast_to([P, A, e - s, C]),
                    MULT,
                )

            # sum over n (partitions via PE, chunks via PSUM accumulation)
            Xf = X.rearrange("p a i c -> p a (i c)")
            ob = outp.tile([1, IRR * C], F32, tag="ob")
            for h in range(2):
                ps = pout.tile([1, 2, 512], F32, tag="ps")
                for kk in range(2):
                    k = h * 2 + kk
                    for a in range(A):
                        nc.tensor.matmul(
                            ps[:, kk, :],
                            ones,
                            Xf[:, a, k * 512:(k + 1) * 512],
                            start=(a == 0),
                            stop=(a == A - 1),
                        )
                nc.scalar.copy(out=ob[:, h * 1024:(h + 1) * 1024],
                               in_=ps.rearrange("m k f -> m (k f)"))
            nc.sync.dma_start(out=out_v[:, b * IRR * C:(b + 1) * IRR * C], in_=ob)

    nc.compile()
    return nc


def kernel(x: np.ndarray, i2l: np.ndarray | None = None) -> np.ndarray:
    x = np.ascontiguousarray(np.asarray(x), dtype=np.float32)
    assert x.shape == (B_FULL, N, IRR, C), x.shape

    if "nc" not in _cache:
        _cache["nc"] = _build_bass()
    nc = _cache["nc"]

    from concourse.bass_utils import run_bass_kernel_spmd

    in_maps = [{"x": x[i * B:(i + 1) * B]} for i in range(N_CORES)]
    res = run_bass_kernel_spmd(nc, in_maps, list(range(N_CORES)))
    out = np.concatenate([res.results[i]["out"] for i in range(N_CORES)], axis=0)
    return out


if __name__ == "__main__":
    xs = np.random.randn(B_FULL, N, IRR, C).astype(np.float32)
    o = kernel(xs)
    print("out", o.shape, o.dtype)


# revision 13
# speedup vs baseline: 1.3725x; 1.0140x over previous
"""CoefficientMaxPool Trainium2 kernel (8-core data-parallel).

Problem: x [32, 512, 16, 128] f32.  Irreps group into degree blocks
l=0:[0,1), l=1:[1,4), l=2:[4,9), l=3:[9,16).  Per (batch, l, channel):
find the neighbor n* maximizing the degree-block squared norm, output
that neighbor's block components -> out [32, 16, 128].

Per core (4 batches), per batch (transposed-norms architecture):
  - DMA x[b] as [p=128, a=4, i=16, c=128] (n = a*128 + p)
  - ACT: x2 = x*x
  - Degree norms land TRANSPOSED in PSUM as NT_l [c, n]:
      l0: PE transposes of x2[:, a, 0, :]
      l2: PE transpose-accumulation over i in [4, 9)
      l1/l3: DVE grouped reduce (n-side) then PE transpose
  - DVE: maxv_l[c] = reduce-max over n (free dim);  maskT_l[c, n] =
    (NT_l == maxv_l) via tensor_scalar is_equal (per-partition scalar)
  - PE: transpose maskT_l back -> MP_l [p(chunk), a, c] in PSUM
  - DVE: x *= MP[l(i)]  in place (winner-select; exact, unique winner)
  - PE: ones[128,1]^T @ x accumulated over a -> out[1, i*c] in PSUM
  - ACT copy PSUM->SBUF, DMA out.
"""

import os
import sys

import numpy as np

for _p in ("/opt/trn_rl_repo", "/opt/pypackages"):
    if _p not in sys.path:
        sys.path.append(_p)

from contextlib import ExitStack

import concourse.bacc as bacc
import concourse.bass as bass
import concourse.tile as tile
from concourse import mybir

N_CORES = 8
B_FULL, N, IRR, C = 32, 512, 16, 128
B = B_FULL // N_CORES  # 4 batches per core
P = 128                # partitions (n within chunk)
A = N // P             # 4 neighbor chunks
BLOCKS = [(0, 1), (1, 4), (4, 9), (9, 16)]  # irrep ranges per degree l
F32 = mybir.dt.float32
ADD = mybir.AluOpType.add
MAX = mybir.AluOpType.max
MULT = mybir.AluOpType.mult
EQ = mybir.AluOpType.is_equal

_cache = {}


def _build_bass():
    nc = bacc.Bacc("TRN2", target_bir_lowering=False, debug=False,
                   num_devices=N_CORES)
    x_in = nc.dram_tensor("x", [B, N, IRR, C], F32, kind="ExternalInput")
    out_t = nc.dram_tensor("out", [B, IRR, C], F32, kind="ExternalOutput")
    ident_d = nc.inline_tensor(np.eye(P, dtype=np.float32), name="ident")

    with tile.TileContext(nc) as tc, ExitStack() as ctx:
        # DRAM view: n = a*P + p  ->  [b, p, a, i, c]
        x_v = x_in.ap().rearrange("b (a p) i c -> b p a i c", p=P)
        out_v = out_t.ap().rearrange("b i c -> (b i c)").unsqueeze(0)

        xp = ctx.enter_context(tc.tile_pool(name="xp", bufs=2))
        x2p = ctx.enter_context(tc.tile_pool(name="x2p", bufs=2))
        med = ctx.enter_context(tc.tile_pool(name="med", bufs=2))
        outp = ctx.enter_context(tc.tile_pool(name="outp", bufs=2))
        singles = ctx.enter_context(tc.tile_pool(name="singles", bufs=1))
        # PSUM: one rotating pool of [128, 512] 1-bank tiles (NT_l / MP_l)
        psc = ctx.enter_context(tc.tile_pool(name="psc", bufs=4, space="PSUM"))
        pout = ctx.enter_context(tc.tile_pool(name="pout", bufs=2, space="PSUM"))

        ones = singles.tile([P, 1], F32)
        nc.vector.memset(ones, 1.0)
        ident = singles.tile([P, P], F32)
        nc.sync.dma_start(out=ident, in_=ident_d.ap())

        for b in range(B):
            X = xp.tile([P, A, IRR, C], F32, tag="X")
            nc.sync.dma_start(out=X, in_=x_v[b])

            X2 = x2p.tile([P, A, IRR, C], F32, tag="X2")
            nc.scalar.activation(X2, X, mybir.ActivationFunctionType.Square)

            # n-side grouped sums for l = 1..3 (DVE)
            norms = med.tile([P, A, 3, C], F32, tag="norms")
            for j, (s, e) in ((0, BLOCKS[1]), (1, BLOCKS[2]), (2, BLOCKS[3])):
                nc.vector.tensor_reduce(
                    out=norms[:, :, j, :],
                    in_=X2[:, :, s:e, :].rearrange("p a i c -> p a c i"),
                    axis=mybir.AxisListType.X, op=ADD)

            # transposed norms NT_l [c, n=a*128+p] in PSUM
            NT = []
            for l in range(4):
                nt = psc.tile([P, A * P], F32, tag="sc")
                NT.append(nt)
            for a in range(A):
                nc.tensor.transpose(NT[0][:, a * P:(a + 1) * P],
                                    X2[:, a, 0, :], ident)
                for l in range(1, 4):
                    nc.tensor.transpose(NT[l][:, a * P:(a + 1) * P],
                                        norms[:, a, l - 1, :], ident)

            # per-l: max over n, equality mask (transposed), transpose back
            MP = []
            for l in range(4):
                maxv = med.tile([P, 1], F32, tag=f"maxv{l}")
                nc.vector.tensor_reduce(
                    out=maxv, in_=NT[l], axis=mybir.AxisListType.X, op=MAX)
                mt = med.tile([P, A * P], F32, tag=f"mt{l}")
                nc.vector.tensor_scalar(
                    out=mt, in0=NT[l], scalar1=maxv, scalar2=None, op0=EQ)
                mp = psc.tile([P, A * P], F32, tag="sc")
                for a in range(A):
                    nc.tensor.transpose(mp[:, a * P:(a + 1) * P],
                                        mt[:, a * P:(a + 1) * P], ident)
                MP.append(mp)  # [p, (a c)]

            # winner-select in place: X *= MP[l(i)]
            for l, (s, e) in enumerate(BLOCKS):
                nc.vector.tensor_tensor(
                    X[:, :, s:e, :],
                    X[:, :, s:e, :],
                    MP[l].rearrange("p (a c) -> p a c", a=A)
                         .unsqueeze(2).broadcast_to([P, A, e - s, C]),
                    MULT,
                )

            # sum over n (partitions via PE, chunks via PSUM accumulation)
            Xf = X.rearrange("p a i c -> p a (i c)")
            ob = outp.tile([1, IRR * C], F32, tag="ob")
            for h in range(2):
                ps = pout.tile([1, 2, 512], F32, tag="ps")
                for kk in range(2):
                    k = h * 2 + kk
                    for a in range(A):
                        nc.tensor.matmul(
                            ps[:, kk, :],
                            ones,
                            Xf[:, a, k * 512:(k + 1) * 512],
                            start=(a == 0),
                            stop=(a == A - 1),
                        )
                nc.scalar.copy(out=ob[:, h * 1024:(h + 1) * 1024],
                               in_=ps.rearrange("m k f -> m (k f)"))
            nc.sync.dma_start(out=out_v[:, b * IRR * C:(b + 1) * IRR * C], in_=ob)

    nc.compile()
    return nc


def kernel(x: np.ndarray, i2l: np.ndarray | None = None) -> np.ndarray:
    x = np.ascontiguousarray(np.asarray(x), dtype=np.float32)
    assert x.shape == (B_FULL, N, IRR, C), x.shape

    if "nc" not in _cache:
        _cache["nc"] = _build_bass()
    nc = _cache["nc"]

    from concourse.bass_utils import run_bass_kernel_spmd

    in_maps = [{"x": x[i * B:(i + 1) * B]} for i in range(N_CORES)]
    res = run_bass_kernel_spmd(nc, in_maps, list(range(N_CORES)))
    out = np.concatenate([res.results[i]["out"] for i in range(N_CORES)], axis=0)
    return out


if __name__ == "__main__":
    xs = np.random.randn(B_FULL, N, IRR, C).astype(np.float32)
    o = kernel(xs)
    print("out", o.shape, o.dtype)


# revision 14
# speedup vs baseline: 1.4100x; 1.0273x over previous
"""CoefficientMaxPool Trainium2 kernel (8-core data-parallel).

Problem: x [32, 512, 16, 128] f32.  Irreps group into degree blocks
l=0:[0,1), l=1:[1,4), l=2:[4,9), l=3:[9,16).  Per (batch, l, channel):
find the neighbor n* maximizing the degree-block squared norm, output
that neighbor's block components -> out [32, 16, 128].

Per core (4 batches), per batch (transposed-norms architecture):
  - DMA x[b] as [p=128, a=4, i=16, c=128] (n = a*128 + p)
  - ACT: x2 = x*x
  - Degree norms land TRANSPOSED in PSUM as NT_l [c, n]:
      l0: PE transposes of x2[:, a, 0, :]
      l2: PE transpose-accumulation over i in [4, 9)
      l1/l3: DVE grouped reduce (n-side) then PE transpose
  - DVE: maxv_l[c] = reduce-max over n (free dim);  maskT_l[c, n] =
    (NT_l == maxv_l) via tensor_scalar is_equal (per-partition scalar)
  - PE: transpose maskT_l back -> MP_l [p(chunk), a, c] in PSUM
  - DVE: x *= MP[l(i)]  in place (winner-select; exact, unique winner)
  - PE: ones[128,1]^T @ x accumulated over a -> out[1, i*c] in PSUM
  - ACT copy PSUM->SBUF, DMA out.
"""

import os
import sys

import numpy as np

for _p in ("/opt/trn_rl_repo", "/opt/pypackages"):
    if _p not in sys.path:
        sys.path.append(_p)

from contextlib import ExitStack

import concourse.bacc as bacc
import concourse.bass as bass
import concourse.tile as tile
from concourse import mybir

N_CORES = 8
B_FULL, N, IRR, C = 32, 512, 16, 128
B = B_FULL // N_CORES  # 4 batches per core
P = 128                # partitions (n within chunk)
A = N // P             # 4 neighbor chunks
BLOCKS = [(0, 1), (1, 4), (4, 9), (9, 16)]  # irrep ranges per degree l
F32 = mybir.dt.float32
ADD = mybir.AluOpType.add
MAX = mybir.AluOpType.max
MULT = mybir.AluOpType.mult
EQ = mybir.AluOpType.is_equal

_cache = {}


def _build_bass():
    nc = bacc.Bacc("TRN2", target_bir_lowering=False, debug=False,
                   num_devices=N_CORES)
    x_in = nc.dram_tensor("x", [B, N, IRR, C], F32, kind="ExternalInput")
    out_t = nc.dram_tensor("out", [B, IRR, C], F32, kind="ExternalOutput")
    ident_d = nc.inline_tensor(np.eye(P, dtype=np.float32), name="ident")

    with tile.TileContext(nc) as tc, ExitStack() as ctx:
        # DRAM view: n = a*P + p  ->  [b, p, a, i, c]
        x_v = x_in.ap().rearrange("b (a p) i c -> b p a i c", p=P)
        out_v = out_t.ap().rearrange("b i c -> (b i c)").unsqueeze(0)

        xp = ctx.enter_context(tc.tile_pool(name="xp", bufs=2))
        x2p = ctx.enter_context(tc.tile_pool(name="x2p", bufs=2))
        med = ctx.enter_context(tc.tile_pool(name="med", bufs=2))
        outp = ctx.enter_context(tc.tile_pool(name="outp", bufs=2))
        singles = ctx.enter_context(tc.tile_pool(name="singles", bufs=1))
        # PSUM: one rotating pool of [128, 512] 1-bank tiles (NT_l / MP_l)
        psc = ctx.enter_context(tc.tile_pool(name="psc", bufs=4, space="PSUM"))
        pout = ctx.enter_context(tc.tile_pool(name="pout", bufs=2, space="PSUM"))

        ones = singles.tile([P, 1], F32)
        nc.vector.memset(ones, 1.0)
        ident = singles.tile([P, P], F32)
        nc.sync.dma_start(out=ident, in_=ident_d.ap())

        for b in range(B):
            X = xp.tile([P, A, IRR, C], F32, tag="X")
            nc.sync.dma_start(out=X, in_=x_v[b])

            X2 = x2p.tile([P, A, IRR, C], F32, tag="X2")
            nc.scalar.activation(X2, X, mybir.ActivationFunctionType.Square)

            # n-side grouped sum for l1 only (DVE); l2/l3 accumulate on PE
            norms1 = med.tile([P, A, C], F32, tag="norms1")
            nc.vector.tensor_reduce(
                out=norms1,
                in_=X2[:, :, 1:4, :].rearrange("p a i c -> p a c i"),
                axis=mybir.AxisListType.X, op=ADD)

            # transposed norms NT_l [c, n=a*128+p] in PSUM
            NT = []
            for l in range(4):
                nt = psc.tile([P, A * P], F32, tag="sc")
                NT.append(nt)
            for a in range(A):
                sl = slice(a * P, (a + 1) * P)
                nc.tensor.transpose(NT[0][:, sl], X2[:, a, 0, :], ident)
                nc.tensor.transpose(NT[1][:, sl], norms1[:, a, :], ident)
                for l in (2, 3):
                    s, e = BLOCKS[l]
                    for i in range(s, e):
                        nc.tensor.matmul(NT[l][:, sl], X2[:, a, i, :], ident,
                                         is_transpose=True,
                                         start=(i == s), stop=(i == e - 1))

            # per-l: max over n, equality mask (transposed), transpose back
            MP = []
            for l in range(4):
                maxv = med.tile([P, 1], F32, tag=f"maxv{l}")
                nc.vector.tensor_reduce(
                    out=maxv, in_=NT[l], axis=mybir.AxisListType.X, op=MAX)
                mt = med.tile([P, A * P], F32, tag=f"mt{l}")
                nc.vector.tensor_scalar(
                    out=mt, in0=NT[l], scalar1=maxv, scalar2=None, op0=EQ)
                mp = psc.tile([P, A * P], F32, tag="sc")
                for a in range(A):
                    nc.tensor.transpose(mp[:, a * P:(a + 1) * P],
                                        mt[:, a * P:(a + 1) * P], ident)
                MP.append(mp)  # [p, (a c)]

            # winner-select in place: X *= MP[l(i)]
            for l, (s, e) in enumerate(BLOCKS):
                nc.vector.tensor_tensor(
                    X[:, :, s:e, :],
                    X[:, :, s:e, :],
                    MP[l].rearrange("p (a c) -> p a c", a=A)
                         .unsqueeze(2).broadcast_to([P, A, e - s, C]),
                    MULT,
                )

            # sum over n (partitions via PE, chunks via PSUM accumulation)
            Xf = X.rearrange("p a i c -> p a (i c)")
            ob = outp.tile([1, IRR * C], F32, tag="ob")
            for h in range(2):
                ps = pout.tile([1, 2, 512], F32, tag="ps")
                for kk in range(2):
                    k = h * 2 + kk
                    for a in range(A):
                        nc.tensor.matmul(
                            ps[:, kk, :],
                            ones,
                            Xf[:, a, k * 512:(k + 1) * 512],
                            start=(a == 0),
                            stop=(a == A - 1),
                        )
                nc.scalar.copy(out=ob[:, h * 1024:(h + 1) * 1024],
                               in_=ps.rearrange("m k f -> m (k f)"))
            nc.sync.dma_start(out=out_v[:, b * IRR * C:(b + 1) * IRR * C], in_=ob)

    nc.compile()
    return nc


def kernel(x: np.ndarray, i2l: np.ndarray | None = None) -> np.ndarray:
    x = np.ascontiguousarray(np.asarray(x), dtype=np.float32)
    assert x.shape == (B_FULL, N, IRR, C), x.shape

    if "nc" not in _cache:
        _cache["nc"] = _build_bass()
    nc = _cache["nc"]

    from concourse.bass_utils import run_bass_kernel_spmd

    in_maps = [{"x": x[i * B:(i + 1) * B]} for i in range(N_CORES)]
    res = run_bass_kernel_spmd(nc, in_maps, list(range(N_CORES)))
    out = np.concatenate([res.results[i]["out"] for i in range(N_CORES)], axis=0)
    return out


if __name__ == "__main__":
    xs = np.random.randn(B_FULL, N, IRR, C).astype(np.float32)
    o = kernel(xs)
    print("out", o.shape, o.dtype)


# revision 15
# speedup vs baseline: 1.4172x; 1.0051x over previous
"""CoefficientMaxPool Trainium2 kernel (8-core data-parallel).

Problem: x [32, 512, 16, 128] f32.  Irreps group into degree blocks
l=0:[0,1), l=1:[1,4), l=2:[4,9), l=3:[9,16).  Per (batch, l, channel):
find the neighbor n* maximizing the degree-block squared norm, output
that neighbor's block components -> out [32, 16, 128].

Per core (4 batches), per batch (transposed-norms architecture):
  - DMA x[b] as [p=128, a=4, i=16, c=128] (n = a*128 + p)
  - ACT: x2 = x*x
  - Degree norms land TRANSPOSED in PSUM as NT_l [c, n]:
      l0: PE transposes of x2[:, a, 0, :]
      l2: PE transpose-accumulation over i in [4, 9)
      l1/l3: DVE grouped reduce (n-side) then PE transpose
  - DVE: maxv_l[c] = reduce-max over n (free dim);  maskT_l[c, n] =
    (NT_l == maxv_l) via tensor_scalar is_equal (per-partition scalar)
  - PE: transpose maskT_l back -> MP_l [p(chunk), a, c] in PSUM
  - DVE: x *= MP[l(i)]  in place (winner-select; exact, unique winner)
  - PE: ones[128,1]^T @ x accumulated over a -> out[1, i*c] in PSUM
  - ACT copy PSUM->SBUF, DMA out.
"""

import os
import sys

import numpy as np

for _p in ("/opt/trn_rl_repo", "/opt/pypackages"):
    if _p not in sys.path:
        sys.path.append(_p)

from contextlib import ExitStack

import concourse.bacc as bacc
import concourse.bass as bass
import concourse.tile as tile
from concourse import mybir

N_CORES = 8
B_FULL, N, IRR, C = 32, 512, 16, 128
B = B_FULL // N_CORES  # 4 batches per core
P = 128                # partitions (n within chunk)
A = N // P             # 4 neighbor chunks
BLOCKS = [(0, 1), (1, 4), (4, 9), (9, 16)]  # irrep ranges per degree l
F32 = mybir.dt.float32
ADD = mybir.AluOpType.add
MAX = mybir.AluOpType.max
MULT = mybir.AluOpType.mult
EQ = mybir.AluOpType.is_equal

_cache = {}


def _build_bass():
    nc = bacc.Bacc("TRN2", target_bir_lowering=False, debug=False,
                   num_devices=N_CORES)
    x_in = nc.dram_tensor("x", [B, N, IRR, C], F32, kind="ExternalInput")
    out_t = nc.dram_tensor("out", [B, IRR, C], F32, kind="ExternalOutput")
    ident_d = nc.inline_tensor(np.eye(P, dtype=np.float32), name="ident")

    with tile.TileContext(nc) as tc, ExitStack() as ctx:
        # DRAM view: n = a*P + p  ->  [b, p, a, i, c]
        x_v = x_in.ap().rearrange("b (a p) i c -> b p a i c", p=P)
        out_v = out_t.ap().rearrange("b i c -> (b i c)").unsqueeze(0)

        xp = ctx.enter_context(tc.tile_pool(name="xp", bufs=2))
        x2p = ctx.enter_context(tc.tile_pool(name="x2p", bufs=2))
        med = ctx.enter_context(tc.tile_pool(name="med", bufs=3))
        outp = ctx.enter_context(tc.tile_pool(name="outp", bufs=2))
        singles = ctx.enter_context(tc.tile_pool(name="singles", bufs=1))
        # PSUM: one rotating pool of [128, 512] 1-bank tiles (NT_l / MP_l)
        psc = ctx.enter_context(tc.tile_pool(name="psc", bufs=4, space="PSUM"))
        pout = ctx.enter_context(tc.tile_pool(name="pout", bufs=2, space="PSUM"))

        ones = singles.tile([P, 1], F32)
        nc.vector.memset(ones, 1.0)
        ident = singles.tile([P, P], F32)
        nc.sync.dma_start(out=ident, in_=ident_d.ap())

        for b in range(B):
            X = xp.tile([P, A, IRR, C], F32, tag="X")
            nc.sync.dma_start(out=X, in_=x_v[b])

            X2 = x2p.tile([P, A, IRR, C], F32, tag="X2")
            nc.scalar.activation(X2, X, mybir.ActivationFunctionType.Square)

            # n-side grouped sum for l1 only (DVE); l2/l3 accumulate on PE
            norms1 = med.tile([P, A, C], F32, tag="norms1")
            nc.vector.tensor_reduce(
                out=norms1,
                in_=X2[:, :, 1:4, :].rearrange("p a i c -> p a c i"),
                axis=mybir.AxisListType.X, op=ADD)

            # transposed norms NT_l [c, n=a*128+p] in PSUM
            NT = []
            for l in range(4):
                nt = psc.tile([P, A * P], F32, tag="sc")
                NT.append(nt)
            for a in range(A):
                sl = slice(a * P, (a + 1) * P)
                nc.tensor.transpose(NT[0][:, sl], X2[:, a, 0, :], ident)
                nc.tensor.transpose(NT[1][:, sl], norms1[:, a, :], ident)
                for l in (2, 3):
                    s, e = BLOCKS[l]
                    for i in range(s, e):
                        nc.tensor.matmul(NT[l][:, sl], X2[:, a, i, :], ident,
                                         is_transpose=True,
                                         start=(i == s), stop=(i == e - 1))

            # per-l: max over n, equality mask (transposed), transpose back
            MP = []
            for l in range(4):
                maxv = med.tile([P, 1], F32, tag=f"maxv{l}")
                nc.vector.tensor_reduce(
                    out=maxv, in_=NT[l], axis=mybir.AxisListType.X, op=MAX)
                mt = med.tile([P, A * P], F32, tag=f"mt{l}")
                nc.vector.tensor_scalar(
                    out=mt, in0=NT[l], scalar1=maxv, scalar2=None, op0=EQ)
                mp = psc.tile([P, A * P], F32, tag="sc")
                for a in range(A):
                    nc.tensor.transpose(mp[:, a * P:(a + 1) * P],
                                        mt[:, a * P:(a + 1) * P], ident)
                MP.append(mp)  # [p, (a c)]

            # winner-select in place: X *= MP[l(i)]
            for l, (s, e) in enumerate(BLOCKS):
                nc.vector.tensor_tensor(
                    X[:, :, s:e, :],
                    X[:, :, s:e, :],
                    MP[l].rearrange("p (a c) -> p a c", a=A)
                         .unsqueeze(2).broadcast_to([P, A, e - s, C]),
                    MULT,
                )

            # sum over n (partitions via PE, chunks via PSUM accumulation)
            Xf = X.rearrange("p a i c -> p a (i c)")
            ob = outp.tile([1, IRR * C], F32, tag="ob")
            for h in range(2):
                ps = pout.tile([1, 2, 512], F32, tag="ps")
                for kk in range(2):
                    k = h * 2 + kk
                    for a in range(A):
                        nc.tensor.matmul(
                            ps[:, kk, :],
                            ones,
                            Xf[:, a, k * 512:(k + 1) * 512],
                            start=(a == 0),
                            stop=(a == A - 1),
                        )
                nc.scalar.copy(out=ob[:, h * 1024:(h + 1) * 1024],
                               in_=ps.rearrange("m k f -> m (k f)"))
            nc.sync.dma_start(out=out_v[:, b * IRR * C:(b + 1) * IRR * C], in_=ob)

    nc.compile()
    return nc


def kernel(x: np.ndarray, i2l: np.ndarray | None = None) -> np.ndarray:
    x = np.ascontiguousarray(np.asarray(x), dtype=np.float32)
    assert x.shape == (B_FULL, N, IRR, C), x.shape

    if "nc" not in _cache:
        _cache["nc"] = _build_bass()
    nc = _cache["nc"]

    from concourse.bass_utils import run_bass_kernel_spmd

    in_maps = [{"x": x[i * B:(i + 1) * B]} for i in range(N_CORES)]
    res = run_bass_kernel_spmd(nc, in_maps, list(range(N_CORES)))
    out = np.concatenate([res.results[i]["out"] for i in range(N_CORES)], axis=0)
    return out


if __name__ == "__main__":
    xs = np.random.randn(B_FULL, N, IRR, C).astype(np.float32)
    o = kernel(xs)
    print("out", o.shape, o.dtype)


# revision 16
# speedup vs baseline: 1.4280x; 1.0077x over previous
"""CoefficientMaxPool Trainium2 kernel (8-core data-parallel).

Problem: x [32, 512, 16, 128] f32.  Irreps group into degree blocks
l=0:[0,1), l=1:[1,4), l=2:[4,9), l=3:[9,16).  Per (batch, l, channel):
find the neighbor n* maximizing the degree-block squared norm, output
that neighbor's block components -> out [32, 16, 128].

Per core (4 batches), per batch (transposed-norms architecture):
  - DMA x[b] as [p=128, a=4, i=16, c=128] (n = a*128 + p)
  - ACT: x2 = x*x
  - Degree norms land TRANSPOSED in PSUM as NT_l [c, n]:
      l0: PE transposes of x2[:, a, 0, :]
      l2: PE transpose-accumulation over i in [4, 9)
      l1/l3: DVE grouped reduce (n-side) then PE transpose
  - DVE: maxv_l[c] = reduce-max over n (free dim);  maskT_l[c, n] =
    (NT_l == maxv_l) via tensor_scalar is_equal (per-partition scalar)
  - PE: transpose maskT_l back -> MP_l [p(chunk), a, c] in PSUM
  - DVE: x *= MP[l(i)]  in place (winner-select; exact, unique winner)
  - PE: ones[128,1]^T @ x accumulated over a -> out[1, i*c] in PSUM
  - ACT copy PSUM->SBUF, DMA out.
"""

import os
import sys

import numpy as np

for _p in ("/opt/trn_rl_repo", "/opt/pypackages"):
    if _p not in sys.path:
        sys.path.append(_p)

from contextlib import ExitStack

import concourse.bacc as bacc
import concourse.bass as bass
import concourse.tile as tile
from concourse import mybir

N_CORES = 8
B_FULL, N, IRR, C = 32, 512, 16, 128
B = B_FULL // N_CORES  # 4 batches per core
P = 128                # partitions (n within chunk)
A = N // P             # 4 neighbor chunks
BLOCKS = [(0, 1), (1, 4), (4, 9), (9, 16)]  # irrep ranges per degree l
F32 = mybir.dt.float32
ADD = mybir.AluOpType.add
MAX = mybir.AluOpType.max
MULT = mybir.AluOpType.mult
EQ = mybir.AluOpType.is_equal

_cache = {}


def _build_bass():
    nc = bacc.Bacc("TRN2", target_bir_lowering=False, debug=False,
                   num_devices=N_CORES)
    x_in = nc.dram_tensor("x", [B, N, IRR, C], F32, kind="ExternalInput")
    out_t = nc.dram_tensor("out", [B, IRR, C], F32, kind="ExternalOutput")
    ident_d = nc.inline_tensor(np.eye(P, dtype=np.float32), name="ident")

    with tile.TileContext(nc) as tc, ExitStack() as ctx:
        # DRAM view: n = a*P + p  ->  [b, p, a, i, c]
        x_v = x_in.ap().rearrange("b (a p) i c -> b p a i c", p=P)
        out_v = out_t.ap().rearrange("b i c -> (b i c)").unsqueeze(0)

        xp = ctx.enter_context(tc.tile_pool(name="xp", bufs=2))
        x2p = ctx.enter_context(tc.tile_pool(name="x2p", bufs=2))
        med = ctx.enter_context(tc.tile_pool(name="med", bufs=3))
        outp = ctx.enter_context(tc.tile_pool(name="outp", bufs=2))
        singles = ctx.enter_context(tc.tile_pool(name="singles", bufs=1))
        # PSUM: one rotating pool of [128, 512] 1-bank tiles (NT_l / MP_l)
        psc = ctx.enter_context(tc.tile_pool(name="psc", bufs=4, space="PSUM"))
        pout = ctx.enter_context(tc.tile_pool(name="pout", bufs=2, space="PSUM"))

        ones = singles.tile([P, 1], F32)
        nc.vector.memset(ones, 1.0)
        ident = singles.tile([P, P], F32)
        nc.sync.dma_start(out=ident, in_=ident_d.ap())

        for b in range(B):
            X = xp.tile([P, A, IRR, C], F32, tag="X")
            X2 = x2p.tile([P, A, IRR, C], F32, tag="X2")
            norms1 = med.tile([P, A, C], F32, tag="norms1")
            # split load/square/l1-sum into a-halves for earlier pipeline start
            for h in range(2):
                ha = slice(2 * h, 2 * h + 2)
                nc.sync.dma_start(out=X[:, ha], in_=x_v[b][:, ha])
                nc.scalar.activation(X2[:, ha], X[:, ha],
                                     mybir.ActivationFunctionType.Square)
                nc.vector.tensor_reduce(
                    out=norms1[:, ha, :],
                    in_=X2[:, ha, 1:4, :].rearrange("p a i c -> p a c i"),
                    axis=mybir.AxisListType.X, op=ADD)

            # transposed norms NT_l [c, n=a*128+p] in PSUM
            NT = []
            for l in range(4):
                nt = psc.tile([P, A * P], F32, tag="sc")
                NT.append(nt)
            for a in range(A):
                sl = slice(a * P, (a + 1) * P)
                nc.tensor.transpose(NT[0][:, sl], X2[:, a, 0, :], ident)
                nc.tensor.transpose(NT[1][:, sl], norms1[:, a, :], ident)
                for l in (2, 3):
                    s, e = BLOCKS[l]
                    for i in range(s, e):
                        nc.tensor.matmul(NT[l][:, sl], X2[:, a, i, :], ident,
                                         is_transpose=True,
                                         start=(i == s), stop=(i == e - 1))

            # per-l: max over n, equality mask (transposed), transpose back
            MP = []
            for l in range(4):
                maxv = med.tile([P, 1], F32, tag=f"maxv{l}")
                nc.vector.tensor_reduce(
                    out=maxv, in_=NT[l], axis=mybir.AxisListType.X, op=MAX)
                mt = med.tile([P, A * P], F32, tag=f"mt{l}")
                nc.vector.tensor_scalar(
                    out=mt, in0=NT[l], scalar1=maxv, scalar2=None, op0=EQ)
                mp = psc.tile([P, A * P], F32, tag="sc")
                for a in range(A):
                    nc.tensor.transpose(mp[:, a * P:(a + 1) * P],
                                        mt[:, a * P:(a + 1) * P], ident)
                MP.append(mp)  # [p, (a c)]

            # winner-select in place: X *= MP[l(i)]  (l2 first: it alone
            # gates the second PE-reduce chunk)
            for l, (s, e) in ((2, BLOCKS[2]), (3, BLOCKS[3]),
                              (0, BLOCKS[0]), (1, BLOCKS[1])):
                nc.vector.tensor_tensor(
                    X[:, :, s:e, :],
                    X[:, :, s:e, :],
                    MP[l].rearrange("p (a c) -> p a c", a=A)
                         .unsqueeze(2).broadcast_to([P, A, e - s, C]),
                    MULT,
                )

            # sum over n (partitions via PE, chunks via PSUM accumulation)
            Xf = X.rearrange("p a i c -> p a (i c)")
            ob = outp.tile([1, IRR * C], F32, tag="ob")
            for h in range(2):
                ps = pout.tile([1, 2, 512], F32, tag="ps")
                for kk in range(2):
                    k = h * 2 + kk
                    for a in range(A):
                        nc.tensor.matmul(
                            ps[:, kk, :],
                            ones,
                            Xf[:, a, k * 512:(k + 1) * 512],
                            start=(a == 0),
                            stop=(a == A - 1),
                        )
                nc.scalar.copy(out=ob[:, h * 1024:(h + 1) * 1024],
                               in_=ps.rearrange("m k f -> m (k f)"))
            nc.sync.dma_start(out=out_v[:, b * IRR * C:(b + 1) * IRR * C], in_=ob)

    nc.compile()
    return nc


def kernel(x: np.ndarray, i2l: np.ndarray | None = None) -> np.ndarray:
    x = np.ascontiguousarray(np.asarray(x), dtype=np.float32)
    assert x.shape == (B_FULL, N, IRR, C), x.shape

    if "nc" not in _cache:
        _cache["nc"] = _build_bass()
    nc = _cache["nc"]

    from concourse.bass_utils import run_bass_kernel_spmd

    in_maps = [{"x": x[i * B:(i + 1) * B]} for i in range(N_CORES)]
    res = run_bass_kernel_spmd(nc, in_maps, list(range(N_CORES)))
    out = np.concatenate([res.results[i]["out"] for i in range(N_CORES)], axis=0)
    return out


if __name__ == "__main__":
    xs = np.random.randn(B_FULL, N, IRR, C).astype(np.float32)
    o = kernel(xs)
    print("out", o.shape, o.dtype)
